# revision 29
# baseline (speedup 1.0000x reference)
"""Trainium2 Bass kernel for nn_ContextualNodeModel (GNN message passing).

Strategy: edge-parallel sharding by destination-node ownership with
host-staged gathers. Nodes are packed into 8 cores x 50 chunks of <=128
nodes by a 5-dimensional balanced bin-packing (node count + per-list
destination degree), so that every (core, chunk) holds <=512 fwd edges,
<=512 bwd edges, <=256 frE and <=256 frL edges -- i.e. tile counts
T=(4,4,2,2) per chunk with ~0% padding.

All endpoint-feature gathers are done on the HOST during input packing:
for each edge slot the concatenated MLP input [x_A(256); x_B(256)] is
staged transposed in DRAM as contiguous [128, 4*ns] blocks per
(chunk, list), so the device only issues one large contiguous DMA per
chunk (plus attr) and the Tensor engine runs back-to-back matmuls:
L1 (5 K-passes incl. attr), ReLU(+b1) on ACT, L2, then segment-sum as a
matmul with a one-hot S built on DVE from per-slot destination indices.
The per-chunk total-flow MLP runs on the aggregated [128-node] block.
No collectives; every core owns its nodes end-to-end.
"""
import os
import sys

sys.path.insert(0, "/opt/trn_rl_repo")

import numpy as np
import ml_dtypes

N_NODES = 50000
N_CORES = 8
CHUNK = 128
N_CHUNKS = 50
D = 256
D_EDGE = 32
D_F = 128
LISTS = ("fwd", "bwd", "frE", "frL")
SEC = {"fwd": 0, "frE": 1, "frL": 1, "bwd": 2}
CAPB = np.array([512, 512, 256, 256, 128], np.float64)
ROUND_TILES = 4

LAST_RESULTS = {}


# ----------------------------------------------------------------- planning
def _assign_nodes(deg):
    """deg [4, N] destination degree per list. Two-level greedy packing:
    nodes -> 8 cores (balance 4 degree sums + count), then per core into
    N_CHUNKS bins under caps (512,512,256,256,128). Returns node_perm
    [8, n_chunks*128] (-1 = empty slot) with n_chunks >= N_CHUNKS equal
    across cores (spill chunks appended if packing overflows)."""
    w = np.concatenate([deg.T, np.ones((N_NODES, 1), np.int32)], 1).astype(np.float64)
    order = np.argsort(-deg.sum(0), kind="stable")
    capc = CAPB * N_CHUNKS
    loads = np.zeros((N_CORES, 5))
    core_of = np.empty(N_NODES, np.int32)
    per_core = N_NODES // N_CORES
    for v in order:
        score = ((loads + w[v]) / capc).max(1)
        score[loads[:, 4] >= per_core] = 1e9
        c = int(np.argmin(score))
        core_of[v] = c
        loads[c] += w[v]

    assign = {}
    max_spill_bins = 0
    for c in range(N_CORES):
        nodes = np.nonzero(core_of == c)[0]
        nd = w[nodes]
        o = np.argsort(-(nd[:, :4] / CAPB[:4]).max(1), kind="stable")
        bl = np.zeros((N_CHUNKS, 5))
        bin_of = np.empty(len(nodes), np.int32)
        spill = []
        for i in o:
            nl = bl + nd[i]
            feas = (nl <= CAPB).all(1)
            if feas.any():
                score = np.where(feas, (nl / CAPB).max(1), 1e9)
                b = int(np.argmin(score))
                bin_of[i] = b
                bl[b] += nd[i]
            else:
                bin_of[i] = -1
                spill.append(i)
        # spill nodes -> extra bins of <=128 nodes
        for k, i in enumerate(spill):
            bin_of[i] = N_CHUNKS + k // CHUNK
        n_spill_bins = (len(spill) + CHUNK - 1) // CHUNK
        max_spill_bins = max(max_spill_bins, n_spill_bins)
        assign[c] = (nodes, bin_of)

    n_chunks = N_CHUNKS + max_spill_bins
    node_perm = np.full((N_CORES, n_chunks * CHUNK), -1, np.int64)
    for c in range(N_CORES):
        nodes, bin_of = assign[c]
        fill = np.zeros(n_chunks, np.int32)
        for v, b in zip(nodes, bin_of):
            node_perm[c, b * CHUNK + fill[b]] = v
            fill[b] += 1
        assert fill.max() <= CHUNK
    return node_perm, n_chunks


def _build_plan(edge_index, same_frame_edge_index):
    ei = np.asarray(edge_index).astype(np.int64)
    fi = np.asarray(same_frame_edge_index).astype(np.int64)
    past, future = ei[0], ei[1]
    early, later = fi[0], fi[1]
    # per list: (A ids, B ids, dst ids, attr table id)
    lists = {"fwd": (future, past, future, 0),
             "bwd": (past, future, past, 0),
             "frE": (early, later, early, 1),
             "frL": (early, later, later, 1)}

    deg = np.stack([np.bincount(lists[L][2], minlength=N_NODES)
                    for L in LISTS]).astype(np.int32)
    node_perm, n_chunks = _assign_nodes(deg)

    node_core = np.empty(N_NODES, np.int32)
    node_slot = np.empty(N_NODES, np.int32)
    for c in range(N_CORES):
        valid = node_perm[c] >= 0
        node_core[node_perm[c][valid]] = c
        node_slot[node_perm[c][valid]] = np.nonzero(valid)[0]

    plan = {"node_perm": node_perm, "n_chunks": n_chunks, "T": {},
            "cores": [dict() for _ in range(N_CORES)]}
    # per-chunk tile counts (max over cores)
    for L in LISTS:
        dst = lists[L][2]
        dc = node_core[dst]
        dchunk = node_slot[dst] // CHUNK
        counts = np.zeros((N_CORES, n_chunks), np.int64)
        np.add.at(counts, (dc, dchunk), 1)
        plan["T"][L] = np.maximum(
            1, (counts.max(axis=0) + CHUNK - 1) // CHUNK).astype(np.int64)

    # chunk-major slot/tile layout (shared by all cores)
    T = plan["T"]
    tiles_per_chunk = np.stack([T[L] for L in LISTS]).sum(0)       # [n_chunks]
    tile_base = np.concatenate([[0], np.cumsum(tiles_per_chunk)])
    slot_base = tile_base * CHUNK
    n_tiles_tot = int(tile_base[-1])
    n_slots_tot = n_tiles_tot * CHUNK
    # per (chunk, list): tile offset of list within chunk, as arrays
    lto = {}
    off = np.zeros(n_chunks, np.int64)
    for L in LISTS:
        lto[L] = off.copy()
        off = off + T[L]
    plan.update(tile_base=tile_base, slot_base=slot_base,
                n_tiles_tot=n_tiles_tot, n_slots_tot=n_slots_tot,
                tiles_per_chunk=tiles_per_chunk, lto=lto)

    # per-core slot assignments
    for c in range(N_CORES):
        a_ids = np.zeros(n_slots_tot, np.int64)
        b_ids = np.zeros(n_slots_tot, np.int64)
        attr_idx = {0: np.full(n_slots_tot, -1, np.int64),
                    1: np.full(n_slots_tot, -1, np.int64)}
        rel = np.full(n_slots_tot, -1.0, np.float32)
        secdeg = np.zeros((n_chunks, CHUNK, 3), np.float32)
        for L in LISTS:
            A, B, dst, ab = lists[L]
            sel = np.nonzero(node_core[dst] == c)[0]
            ds = node_slot[dst[sel]]
            ch = ds // CHUNK
            o = np.argsort(ch, kind="stable")
            sel, ch, ds = sel[o], ch[o], ds[o]
            within = np.arange(len(sel), dtype=np.int64)
            if len(sel):
                brk = np.nonzero(np.diff(ch))[0] + 1
                starts = np.concatenate([[0], brk])
                lens = np.diff(np.concatenate([starts, [len(sel)]]))
                within -= np.repeat(starts, lens)
            pos = slot_base[ch] + lto[L][ch] * CHUNK + within
            a_ids[pos] = A[sel]
            b_ids[pos] = B[sel]
            attr_idx[ab][pos] = sel
            rel[pos] = (ds % CHUNK).astype(np.float32)
            np.add.at(secdeg, (ch, ds % CHUNK, SEC[L]), 1.0)
        plan["cores"][c] = {"a_ids": a_ids, "b_ids": b_ids,
                            "attr_idx": attr_idx, "rel": rel, "secdeg": secdeg}
    return plan


# ----------------------------------------------------------- input packing
def _pack_shared_weights(inputs):
    bf16 = ml_dtypes.bfloat16
    d = {}
    W1 = {"fwd": inputs["Wf1"], "bwd": inputs["Wb1"],
          "frE": inputs["Wr1"], "frL": inputs["Wr1"]}
    W2 = {"fwd": inputs["Wf2"], "bwd": inputs["Wb2"],
          "frE": inputs["Wr2"], "frL": inputs["Wr2"]}
    b1 = {"fwd": inputs["bf1"], "bwd": inputs["bb1"],
          "frE": inputs["br1"], "frL": inputs["br1"]}
    b2 = {"fwd": inputs["bf2"], "bwd": inputs["bb2"],
          "frE": inputs["br2"], "frL": inputs["br2"]}

    wk = np.zeros((128, 4 * 4 * 256), np.float32)
    wa = np.zeros((D_EDGE, 4 * 256), np.float32)
    w2 = np.zeros((128, 4 * 2 * 128), np.float32)
    b1p = np.zeros((128, 8), np.float32)
    for i, L in enumerate(LISTS):
        Wf = np.asarray(W1[L], np.float32)          # [544, 256]
        for kb in range(4):
            wk[:, (i * 4 + kb) * 256:(i * 4 + kb + 1) * 256] = \
                Wf[kb * 128:(kb + 1) * 128]
        wa[:, i * 256:(i + 1) * 256] = Wf[512:544]
        Ws = np.asarray(W2[L], np.float32)          # [256, 128]
        for hb in range(2):
            w2[:, (i * 2 + hb) * 128:(i * 2 + hb + 1) * 128] = \
                Ws[hb * 128:(hb + 1) * 128]
        bb = np.asarray(b1[L], np.float32)
        for hb in range(2):
            b1p[:, i * 2 + hb] = bb[hb * 128:(hb + 1) * 128]
    d["Wk"] = wk.astype(bf16)
    d["Watt"] = wa.astype(bf16)
    d["W2"] = w2.astype(bf16)
    d["b1"] = b1p

    wt1 = np.zeros((128, 3 * 512), np.float32)
    Wt1 = np.asarray(inputs["Wt1"], np.float32)     # [384, 512]
    for kb in range(3):
        wt1[:, kb * 512:(kb + 1) * 512] = Wt1[kb * 128:(kb + 1) * 128]
    d["Wt1"] = wt1.astype(bf16)
    wt2 = np.zeros((128, 4 * 256), np.float32)
    Wt2 = np.asarray(inputs["Wt2"], np.float32)     # [512, 256]
    for hb in range(4):
        wt2[:, hb * 256:(hb + 1) * 256] = Wt2[hb * 128:(hb + 1) * 128]
    d["Wt2"] = wt2.astype(bf16)
    bt1p = np.zeros((128, 4), np.float32)
    bt1 = np.asarray(inputs["bt1"], np.float32)
    for hb in range(4):
        bt1p[:, hb] = bt1[hb * 128:(hb + 1) * 128]
    d["bt1"] = bt1p
    d["bt2bc"] = np.tile(np.asarray(inputs["bt2"], np.float32)[None, :],
                         (128, 1))
    # b2 per section (frame shared) for the aggregation bias
    d["_b2sec"] = np.stack([np.asarray(b2["fwd"], np.float32),
                            np.asarray(b2["frE"], np.float32),
                            np.asarray(b2["bwd"], np.float32)])   # [3, 128]
    d["_use_bias_agg"] = bool(np.abs(d["_b2sec"]).max() > 0)
    return d


def _pack_core_inputs(xbf, attr_ext, plan, shared, c):
    bf16 = ml_dtypes.bfloat16
    cp = plan["cores"][c]
    n_chunks = plan["n_chunks"]
    T = plan["T"]
    n_slots = plan["n_slots_tot"]
    n_tiles = plan["n_tiles_tot"]
    slot_base = plan["slot_base"]
    lto = plan["lto"]

    d = {k: v for k, v in shared.items() if not k.startswith("_")}

    XA = xbf[cp["a_ids"]]                            # [S, 256] bf16
    XB = xbf[cp["b_ids"]]
    XCAT = np.concatenate([XA, XB], axis=1).reshape(n_slots, 4, 128)
    at0 = attr_ext[0][np.where(cp["attr_idx"][0] >= 0, cp["attr_idx"][0],
                               attr_ext[0].shape[0] - 1)]
    at1 = attr_ext[1][np.where(cp["attr_idx"][1] >= 0, cp["attr_idx"][1],
                               attr_ext[1].shape[0] - 1)]
    ATV = np.where((cp["attr_idx"][0] >= 0)[:, None], at0,
                   np.where((cp["attr_idx"][1] >= 0)[:, None], at1,
                            np.zeros((1, D_EDGE), at0.dtype)))    # [S, 32]

    XT = np.empty((128, 4 * n_slots), bf16)
    AT = np.empty((D_EDGE, n_slots), bf16)
    for ch in range(n_chunks):
        for L in LISTS:
            ns = int(T[L][ch]) * CHUNK
            s0 = int(slot_base[ch]) + int(lto[L][ch]) * CHUNK
            XT[:, 4 * s0:4 * s0 + 4 * ns] = \
                XCAT[s0:s0 + ns].transpose(2, 1, 0).reshape(128, 4 * ns)
            AT[:, s0:s0 + ns] = ATV[s0:s0 + ns].T
    d["XT"] = XT
    d["AT"] = np.ascontiguousarray(AT)
    d["rel"] = np.ascontiguousarray(
        cp["rel"].reshape(n_tiles, CHUNK).T).astype(bf16)          # [128, n_tiles]

    b2sec = shared["_b2sec"]                         # [3, 128]
    if shared["_use_bias_agg"]:
        # bias_agg[ch, f, sec, n] = b2sec[sec, f] * secdeg[ch, n, sec]
        bia = (b2sec.T[None, :, :, None] *
               cp["secdeg"].transpose(0, 2, 1)[:, None, :, :]).astype(np.float32)
        d["bias_agg"] = np.ascontiguousarray(bia)    # [n_chunks, 128, 3, 128]
    tmax = int(plan["tiles_per_chunk"].max())
    iota = np.tile(np.arange(CHUNK, dtype=np.float32)[None, None, :],
                   (128, tmax, 1))
    d["iota"] = iota.astype(bf16)                    # [128, tmax, 128]
    return d


# ------------------------------------------------------------ bass program
def _build_bass(plan, shapes, use_bias_agg):
    import concourse.bacc as bacc
    import concourse.tile as tile
    import concourse.mybir as mybir

    bf = mybir.dt.bfloat16
    f32 = mybir.dt.float32

    n_chunks = plan["n_chunks"]
    T = plan["T"]
    slot_base = plan["slot_base"]
    tile_base = plan["tile_base"]
    tiles_per_chunk = plan["tiles_per_chunk"]
    lto = plan["lto"]

    debug = bool(int(os.environ.get("GNN_DEBUG_DUMP", "0")))
    nc = bacc.Bacc("TRN2", target_bir_lowering=False)
    dr = {}
    for name, (shape, dt) in shapes.items():
        kind = "ExternalOutput" if name == "out" else "ExternalInput"
        dr[name] = nc.dram_tensor(name, list(shape), dt, kind=kind)
    if debug:
        dr["dbg_hTs"] = nc.dram_tensor("dbg_hTs", [4, 128, 2, 512], bf,
                                       kind="ExternalOutput")
        dr["dbg_Fs"] = nc.dram_tensor("dbg_Fs", [4, 128, 512], bf,
                                      kind="ExternalOutput")
        dr["dbg_agg"] = nc.dram_tensor("dbg_agg", [128, 3, 128], bf,
                                       kind="ExternalOutput")

    with tile.TileContext(nc) as tc:
        with (
            tc.tile_pool(name="const", bufs=1) as cpool,
            tc.tile_pool(name="gx", bufs=3) as gxpool,
            tc.tile_pool(name="work", bufs=2) as wpool,
            tc.tile_pool(name="ps_hT", bufs=1, space="PSUM") as ps_hT,
            tc.tile_pool(name="ps_F", bufs=2, space="PSUM") as ps_F,
            tc.tile_pool(name="ps_agg", bufs=1, space="PSUM") as ps_agg,
            tc.tile_pool(name="ps_m2", bufs=1, space="PSUM") as ps_m2,
        ):
            def cload(name, dt):
                t = cpool.tile(list(shapes[name][0]), dt, tag=name)
                nc.scalar.dma_start(t[:], dr[name][:])
                return t

            rel_sb = cload("rel", bf)
            Wk_sb = cload("Wk", bf)
            Watt_sb = cload("Watt", bf)
            W2_sb = cload("W2", bf)
            b1_sb = cload("b1", f32)
            Wt1_sb = cload("Wt1", bf)
            Wt2_sb = cload("Wt2", bf)
            bt1_sb = cload("bt1", f32)
            bt2bc_sb = cload("bt2bc", f32)
            iota_sb = cload("iota", bf)

            li = {L: i for i, L in enumerate(LISTS)}

            for ch in range(n_chunks):
                tch = int(tiles_per_chunk[ch])
                ns_ch = tch * CHUNK
                s0 = int(slot_base[ch])
                t0 = int(tile_base[ch])

                xt = gxpool.tile([128, 4 * ns_ch], bf, tag="xt")
                nc.sync.dma_start(xt[:], dr["XT"][:, 4 * s0:4 * (s0 + ns_ch)])
                at = gxpool.tile([32, ns_ch], bf, tag="at")
                nc.scalar.dma_start(at[:], dr["AT"][:, s0:s0 + ns_ch])
                if use_bias_agg:
                    bia = gxpool.tile([128, 3, 128], f32, tag="bia")
                    nc.scalar.dma_start(bia[:], dr["bias_agg"][ch])

                # one-hot S for every tile of this chunk in one DVE op
                Sall = wpool.tile([128, tch, 128], bf, tag="Sall")
                nc.vector.tensor_tensor(
                    out=Sall[:],
                    in0=rel_sb[:, t0:t0 + tch].to_broadcast([128, tch, 128]),
                    in1=iota_sb[:, :tch, :],
                    op=mybir.AluOpType.is_equal)

                aggT = ps_agg.tile([128, 3, 128], f32, tag="aggT")
                n_sec_tiles = {0: int(T["fwd"][ch]),
                               1: int(T["frE"][ch] + T["frL"][ch]),
                               2: int(T["bwd"][ch])}
                sec_first = {0: True, 1: True, 2: True}
                sec_done = {0: 0, 1: 0, 2: 0}

                for L in LISTS:
                    iL = li[L]
                    Tc = int(T[L][ch])
                    ns = Tc * CHUNK
                    loff = int(lto[L][ch]) * CHUNK
                    xoff = 4 * loff
                    sec = SEC[L]

                    for r0 in range(0, Tc, ROUND_TILES):
                        rt = min(ROUND_TILES, Tc - r0)
                        rn = rt * CHUNK
                        e0 = r0 * CHUNK
                        hTh = [ps_hT.tile([128, 512], f32, tag=f"hT{hb}",
                                          name=f"hT{hb}")
                               for hb in range(2)]
                        hTs = [wpool.tile([128, 512], bf, tag=f"hTs{hb}",
                                          name=f"hTs{hb}")
                               for hb in range(2)]
                        for hb in range(2):
                            for kb in range(4):
                                nc.tensor.matmul(
                                    hTh[hb][:, :rn],
                                    Wk_sb[:, (iL * 4 + kb) * 256 + hb * 128:
                                          (iL * 4 + kb) * 256 + hb * 128 + 128],
                                    xt[:, xoff + kb * ns + e0:
                                       xoff + kb * ns + e0 + rn],
                                    start=(kb == 0), stop=False)
                            nc.tensor.matmul(
                                hTh[hb][:, :rn],
                                Watt_sb[:, iL * 256 + hb * 128:
                                        iL * 256 + hb * 128 + 128],
                                at[:, loff + e0:loff + e0 + rn],
                                start=False, stop=True)
                            if hb == 0:
                                nc.scalar.activation(
                                    hTs[hb][:, :rn], hTh[hb][:, :rn],
                                    mybir.ActivationFunctionType.Relu,
                                    bias=b1_sb[:, iL * 2 + hb:iL * 2 + hb + 1])
                            else:
                                nc.vector.tensor_scalar(
                                    out=hTs[hb][:, :rn], in0=hTh[hb][:, :rn],
                                    scalar1=b1_sb[:, iL * 2 + hb:iL * 2 + hb + 1],
                                    scalar2=0.0,
                                    op0=mybir.AluOpType.add,
                                    op1=mybir.AluOpType.max)
                            if debug and ch == 0 and r0 == 0:
                                nc.sync.dma_start(
                                    dr["dbg_hTs"][iL, :, hb, :rn],
                                    hTs[hb][:, :rn])
                        Fp = ps_F.tile([128, 512], f32, tag="F")
                        for i in range(rt):
                            for hb in range(2):
                                nc.tensor.matmul(
                                    Fp[:, i * 128:(i + 1) * 128],
                                    hTs[hb][:, i * 128:(i + 1) * 128],
                                    W2_sb[:, (iL * 2 + hb) * 128:
                                          (iL * 2 + hb + 1) * 128],
                                    start=(hb == 0), stop=(hb == 1))
                        Fs = wpool.tile([128, 512], bf, tag="Fs")
                        if iL % 2 == 0:
                            nc.scalar.activation(
                                Fs[:, :rn], Fp[:, :rn],
                                mybir.ActivationFunctionType.Copy)
                        else:
                            nc.vector.tensor_copy(out=Fs[:, :rn],
                                                  in_=Fp[:, :rn])
                        if debug and ch == 0 and r0 == 0:
                            nc.sync.dma_start(dr["dbg_Fs"][iL, :, :rn],
                                              Fs[:, :rn])
                        for i in range(rt):
                            tloc = int(lto[L][ch]) + r0 + i
                            first = sec_first[sec]
                            sec_first[sec] = False
                            sec_done[sec] += 1
                            nc.tensor.matmul(
                                aggT[:, sec, :],
                                Fs[:, i * 128:(i + 1) * 128],
                                Sall[:, tloc, :],
                                start=first,
                                stop=(sec_done[sec] == n_sec_tiles[sec]))

                # ---- total-flow MLP, batched over chunk pairs
                half = ch % 2
                if half == 0:
                    aggTs2 = wpool.tile([128, 3, 256], bf, tag="aggTs2",
                                        name="aggTs2")
                dstT = aggTs2[:, :, half * 128:(half + 1) * 128]
                if use_bias_agg:
                    nc.vector.tensor_tensor(out=dstT, in0=aggT[:],
                                            in1=bia[:],
                                            op=mybir.AluOpType.add)
                else:
                    nc.vector.tensor_copy(out=dstT, in_=aggT[:])
                if debug and ch == 0:
                    nc.sync.dma_start(dr["dbg_agg"][:],
                                      aggTs2[:, :, 0:128])
                if half == 1 or ch == n_chunks - 1:
                    W = (half + 1) * 128
                    h2 = ps_m2.tile([128, 4, 256], f32, tag="h2")
                    for hb in range(4):
                        for kb in range(3):
                            nc.tensor.matmul(
                                h2[:, hb, :W],
                                Wt1_sb[:, kb * 512 + hb * 128:
                                       kb * 512 + hb * 128 + 128],
                                aggTs2[:, kb, :W],
                                start=(kb == 0), stop=(kb == 2))
                    h2s = wpool.tile([128, 4, 256], bf, tag="h2s")
                    for hb in range(4):
                        nc.scalar.activation(
                            h2s[:, hb, :W], h2[:, hb, :W],
                            mybir.ActivationFunctionType.Relu,
                            bias=bt1_sb[:, hb:hb + 1])
                    op2 = ps_m2.tile([128, 2, 256], f32, tag="op2")
                    for h2i in range(half + 1):
                        for hb in range(4):
                            nc.tensor.matmul(
                                op2[:, h2i, :],
                                h2s[:, hb, h2i * 128:(h2i + 1) * 128],
                                Wt2_sb[:, hb * 256:(hb + 1) * 256],
                                start=(hb == 0), stop=(hb == 3))
                        outs = wpool.tile([128, 256], f32, tag="outs")
                        nc.vector.tensor_tensor(out=outs[:],
                                                in0=op2[:, h2i, :],
                                                in1=bt2bc_sb[:],
                                                op=mybir.AluOpType.add)
                        nc.sync.dma_start(dr["out"][ch - half + h2i],
                                          outs[:])

    nc.compile()
    return nc


# ----------------------------------------------------------------- kernel
def kernel(**inputs):
    import concourse.mybir as mybir
    from concourse.bass_utils import run_bass_kernel_spmd

    bf = mybir.dt.bfloat16
    f32 = mybir.dt.float32
    bf16 = ml_dtypes.bfloat16

    plan = _build_plan(np.asarray(inputs["edge_index"]),
                       np.asarray(inputs["same_frame_edge_index"]))
    shared = _pack_shared_weights(inputs)
    xbf = np.asarray(inputs["x"], np.float32).astype(bf16)
    attr_ext = {
        0: np.vstack([np.asarray(inputs["edge_attr"], np.float32),
                      np.zeros((1, D_EDGE), np.float32)]).astype(bf16),
        1: np.vstack([np.asarray(inputs["same_frame_edge_attr"], np.float32),
                      np.zeros((1, D_EDGE), np.float32)]).astype(bf16),
    }
    cores = [_pack_core_inputs(xbf, attr_ext, plan, shared, c)
             for c in range(N_CORES)]

    shapes = {}
    for name, arr in cores[0].items():
        dt = {np.dtype(np.float32): f32,
              np.dtype(bf16): bf}[arr.dtype]
        shapes[name] = (arr.shape, dt)
    shapes["out"] = ((plan["n_chunks"], 128, 256), f32)

    nc = _build_bass(plan, shapes, shared["_use_bias_agg"])

    trace = bool(int(os.environ.get("GNN_TRACE", "0")))
    res = run_bass_kernel_spmd(nc, cores, core_ids=list(range(N_CORES)),
                               trace=trace)
    LAST_RESULTS["res"] = res

    out = np.zeros((N_NODES, 256), np.float32)
    for c in range(N_CORES):
        oc = np.asarray(res.results[c]["out"], np.float32).reshape(-1, 256)
        valid = plan["node_perm"][c] >= 0
        out[plan["node_perm"][c][valid]] = oc[valid]
    return out
